# revision 25
# baseline (speedup 1.0000x reference)
"""Trainium2 Bass kernel for nn_GATLayered (graph transformer w/ edge features).

Contract: kernel(**inputs) takes FULL numpy inputs (as produced by the
problem's setup_inputs), distributes across 8 NeuronCores, and returns the
FULL [B, N, D] float32 output.

Key algebraic restructure vs the reference:
  - E = edge_emb[edge_types] ([B,N,N,D], 256MB) is never materialized.
    Since edge_types has only ET=16 values, Ke = E @ Wk + bk collapses to a
    [16, D] table; the per-(i,j) edge score becomes a gather of
    qe[i,h,t] = Q_i . ke_table[t, head h], implemented as a one-hot matmul
    that accumulates straight into the attention-score PSUM.
  - adj masking is a rank-16 matmul (rowrep @ maskbias) accumulated into the
    same PSUM: scores never leave PSUM before the softmax exp.
  - Sharding: core c handles graph b=c//2; layer 0 is computed full-graph on
    both cores of a pair (redundant), layer 1 is query-sharded (128 rows per
    core). Cores never communicate.
  - All weights/constants are packed host-side into a few wide [128, X]
    tensors so each lands with ONE descriptor-light DMA (128 descriptors of
    multi-KB rows instead of thousands of 512B rows), and the issue load is
    spread across the scalar/vector/tensor/sync queues so no single engine
    serializes the load phase.
"""

import os
import sys

import numpy as np

for _p in ("/opt/trn_rl_repo", "/root/.axon_site/_ro/trn_rl_repo"):
    if os.path.isdir(_p) and _p not in sys.path:
        sys.path.insert(0, _p)

import ml_dtypes

import concourse.bacc as bacc
import concourse.bass as bass
import concourse.mybir as mybir
import concourse.tile as tile
from concourse.bass_utils import run_bass_kernel_spmd

BF16 = mybir.dt.bfloat16
F32 = mybir.dt.float32
I32 = mybir.dt.int32
AF = mybir.ActivationFunctionType
OP = mybir.AluOpType

B, N, D, H, L, I = 4, 256, 256, 8, 2, 1024
V, ET, MAXPOS = 32000, 16, 512
DH = D // H
SCALE = 1.0 / float(np.sqrt(DH))
N_CORES = 8
KD = D // 128          # 2 contraction tiles over D
FT = I // 128          # 8 tiles over FFN dim

bf16 = ml_dtypes.bfloat16
DIAG_FOLD = os.environ.get("DIAG_FOLD", "0") == "1"
QE_BATCH = os.environ.get("QE_BATCH", "1") == "1"

# ---- packed-tensor column offsets (element units of the pack's dtype) ----
# wA (bf16): [wq0 | wk0 | wv0 | eeT]            width 1568
A_WQ, A_WK, A_WV, A_EE = 0, 512, 1024, 1536
WA_W = 1568
# wB (bf16): [wo0 | w1_0 | w2_0]                width 4608
B_WO, B_W1, B_W2 = 0, 512, 2560
WB_W = 4608
# wC (bf16): [wq1 | wk1 | wv1 | wo1 | w1_1 | w2_1]  width 6144
C_WQ, C_WK, C_WV, C_WO, C_W1, C_W2 = 0, 512, 1024, 1536, 2048, 4096
WC_W = 6144
# cst (f32): [bq | bk | b1T | idf]              width 152 (tiny, lands first)
F_BQ, F_BK, F_B1T, F_IDF = 0, 4, 8, 24
CST_W = 152
LNR_W = L * 4 * D       # ln params, replicated across partitions (separate, late)
# cbf (bf16): [hm | bdm | idb]                  width 272
X_HM, X_BDM, X_IDB = 0, 16, 144
CBF_W = 272
# m16 (bf16, 16 partitions): [mbr | rr]         width 4224
M_MBR, M_RR = 0, 4096
M16_W = 4224
OH_W = (N // 8) * 256   # onehot width 8192


def _ap(t, offset, dims):
    """Hand-built access pattern on a Tile tile or tensor handle."""
    h = t.tensor if hasattr(t, "tensor") else t
    return bass.AP(h, offset, dims)


class TV:
    """Column-window view over a wide SBUF tile: behaves like a [P, w] tile."""

    def __init__(self, t, c0, w):
        self.t, self.c0, self.w = t, c0, w

    def __getitem__(self, idx):
        if idx == slice(None):
            return self.t[:, self.c0:self.c0 + self.w]
        ps, cs = idx
        a = self.c0 if cs.start is None else self.c0 + cs.start
        b = self.c0 + self.w if cs.stop is None else self.c0 + cs.stop
        return self.t[ps, a:b]


def build_nc():
    nc = bacc.Bacc("TRN2", target_bir_lowering=False, debug=False,
                   num_devices=N_CORES)

    # ---------------- DRAM tensors ----------------
    # x0 = tok_emb[word_ids] + pos_emb, gathered host-side (keeps the gpsimd
    # indirect-DMA + drain off the critical path).
    x0_d = nc.dram_tensor("x0", [2, 128, D], F32, kind="ExternalInput")
    wA_d = nc.dram_tensor("wA", [128, WA_W], BF16, kind="ExternalInput")
    wB_d = nc.dram_tensor("wB", [128, WB_W], BF16, kind="ExternalInput")
    wC_d = nc.dram_tensor("wC", [128, WC_W], BF16, kind="ExternalInput")
    cst_d = nc.dram_tensor("cst", [128, CST_W], F32, kind="ExternalInput")
    lnr_d = nc.dram_tensor("lnr", [128, LNR_W], F32, kind="ExternalInput")
    cbf_d = nc.dram_tensor("cbf", [128, CBF_W], BF16, kind="ExternalInput")
    oh_d = nc.dram_tensor("oh", [128, OH_W], BF16, kind="ExternalInput")
    m16_d = nc.dram_tensor("m16", [16, M16_W], BF16, kind="ExternalInput")
    boeff_d = nc.dram_tensor("boeff", [L, D], BF16, kind="ExternalInput")
    b2r_d = nc.dram_tensor("b2r", [L, D], BF16, kind="ExternalInput")
    out_d = nc.dram_tensor("out", [128, D], F32, kind="ExternalOutput")

    with tile.TileContext(nc) as tc:
        with tc.tile_pool(name="pc", bufs=1) as pc, \
             tc.tile_pool(name="st", bufs=1) as st, \
             tc.tile_pool(name="pw", bufs=3) as pw, \
             tc.tile_pool(name="pps", bufs=2, space="PSUM") as pps, \
             tc.tile_pool(name="ppq", bufs=2, space="PSUM") as ppq, \
             tc.tile_pool(name="ppt", bufs=2, space="PSUM") as ppt, \
             tc.tile_pool(name="ppa", bufs=2, space="PSUM") as ppa:

            # ---------------- packed SBUF residents ----------------
            wA_sb = pc.tile([128, WA_W], BF16)
            wB_sb = pc.tile([128, WB_W], BF16)
            wC_sb = pc.tile([128, WC_W], BF16)
            cst_sb = pc.tile([128, CST_W], F32)
            lnr_sb = pc.tile([128, LNR_W], F32)
            cbf_sb = pc.tile([128, CBF_W], BF16)
            oh_sb = pc.tile([128, OH_W], BF16)
            m16_sb = pc.tile([16, M16_W], BF16)
            boeff_sb = [pc.tile([1, D], BF16, tag=f"boe{l}", name=f"boe{l}") for l in range(L)]
            b2r_sb = [pc.tile([1, D], BF16, tag=f"b2r{l}", name=f"b2r{l}") for l in range(L)]

            # ---- x0 first: already gathered host-side, one small DMA/tile ----
            x_nat = [st.tile([128, D], F32, tag=f"x{it}", name=f"x{it}") for it in range(2)]
            for it in range(2):
                nc.sync.dma_start(x_nat[it][:], x0_d[it, :, :])

            # ---- bulk loads, spread across the DMA-capable queues by first use
            # (only SP/Activation HWDGE + gpsimd can issue DMAs on trn2) ----
            # Only SP/Activation have hardware DGE; gpsimd DMA is slow ucode
            # descriptor generation — keep ALL bulk loads on the two HW queues.
            nc.sync.dma_start(cst_sb[:], cst_d[:])        # idf/biases: tiny, first
            nc.sync.dma_start(cbf_sb[:], cbf_d[:])        # hm/bdm/idb
            nc.sync.dma_start(m16_sb[:], m16_d[:])        # mask bias + rowrep
            nc.scalar.dma_start(wA_sb[:], wA_d[:])        # L0 qkv weights
            # oh (2MB) gates the first attention scores (~10us in).
            OHH = OH_W // 2
            nc.scalar.dma_start(oh_sb[:, 0:OHH], oh_d[:, 0:OHH])
            nc.sync.dma_start(oh_sb[:, OHH:OH_W], oh_d[:, OHH:OH_W])
            for l in range(L):
                nc.sync.dma_start(boeff_sb[l][:], boeff_d[l:l + 1, :])
                nc.sync.dma_start(b2r_sb[l][:], b2r_d[l:l + 1, :])
            nc.scalar.dma_start(wB_sb[:], wB_d[:])        # L0 wo/ffn (~mid L0)
            nc.sync.dma_start(lnr_sb[:], lnr_d[:])        # ln params (~mid L0)
            WCH = WC_W // 2
            nc.scalar.dma_start(wC_sb[:, 0:WCH], wC_d[:, 0:WCH])      # L1 (late)
            nc.sync.dma_start(wC_sb[:, WCH:WC_W], wC_d[:, WCH:WC_W])

            # ---------------- views into the packs ----------------
            wq_sb = [[TV(wA_sb, A_WQ + 256 * k, 256) for k in range(KD)],
                     [TV(wC_sb, C_WQ + 256 * k, 256) for k in range(KD)]]
            wk_sb = [[TV(wA_sb, A_WK + 256 * k, 256) for k in range(KD)],
                     [TV(wC_sb, C_WK + 256 * k, 256) for k in range(KD)]]
            wv_sb = [[TV(wA_sb, A_WV + 256 * k, 256) for k in range(KD)],
                     [TV(wC_sb, C_WV + 256 * k, 256) for k in range(KD)]]
            wo_sb = [[TV(wA_sb, 0, 0), TV(wA_sb, 0, 0)],  # unused slots
                     [TV(wC_sb, C_WO + 256 * k, 256) for k in range(KD)]]
            wo_sb[0] = [TV(wB_sb, B_WO + 256 * k, 256) for k in range(KD)]
            w1_sb = [[TV(wB_sb, B_W1 + 1024 * k, 1024) for k in range(KD)],
                     [TV(wC_sb, C_W1 + 1024 * k, 1024) for k in range(KD)]]
            w2_sb = [[TV(wB_sb, B_W2 + 256 * f, 256) for f in range(FT)],
                     [TV(wC_sb, C_W2 + 256 * f, 256) for f in range(FT)]]
            eeT_sb = [TV(wA_sb, A_EE + ET * k, ET) for k in range(KD)]
            bq_sb = TV(cst_sb, F_BQ, L * KD)
            bk_sb = TV(cst_sb, F_BK, L * KD)
            b1T_sb = TV(cst_sb, F_B1T, L * FT)
            idf_sb = TV(cst_sb, F_IDF, 128)
            ln_sb = [[TV(lnr_sb, (l * 4 + k) * D, D) for k in range(4)] for l in range(L)]
            bdm_sb = TV(cbf_sb, X_BDM, 128)
            idb_sb = TV(cbf_sb, X_IDB, 128)
            mbr_sb = TV(m16_sb, M_MBR, 4096)
            rr_sb = TV(m16_sb, M_RR, 128)
            cbf_pitch = cbf_sb[:].ap[0][0]

            ones_sb = pc.tile([1, 128], BF16)
            nc.vector.memset(ones_sb[:], 1.0)
            eps_sb = pc.tile([128, 1], F32)
            nc.vector.memset(eps_sb[:], 1e-5)

            # ---------------- layers ----------------
            for l in range(L):
                n_q = N if l == 0 else 128
                n_it = n_q // 128      # query row-tiles
                NG = n_q // 16         # score groups

                # transpose x -> xT (bf16) [KD][128, N]
                xT = [st.tile([128, N], BF16, tag=f"xT{k}", name=f"xT{k}") for k in range(KD)]
                for k in range(KD):
                    for it in range(2):
                        tp = ppt.tile([128, 128], F32, tag="tp", name="tp")
                        nc.tensor.transpose(tp[:], x_nat[it][:, 128 * k:128 * (k + 1)], idf_sb[:])
                        nc.scalar.copy(xT[k][:, 128 * it:128 * (it + 1)], tp[:])

                # K^T [nd][128, N] and V [jt][128, D], Qhm [nd][128, n_q*8]
                KT = [st.tile([128, N], BF16, tag=f"KT{k}", name=f"KT{k}") for k in range(KD)]
                for nt in range(KD):
                    ps = ppa.tile([128, N], F32, tag="acc", name="acc")
                    for k in range(KD):
                        nc.tensor.matmul(ps[:], wk_sb[l][k][:, 128 * nt:128 * (nt + 1)],
                                         xT[k][:], start=(k == 0), stop=(k == KD - 1))
                    nc.vector.tensor_scalar(out=KT[nt][:], in0=ps[:],
                                            scalar1=bk_sb[:, l * KD + nt:l * KD + nt + 1],
                                            scalar2=None, op0=OP.add)
                Vn = [st.tile([128, D], BF16, tag=f"V{j}", name=f"V{j}") for j in range(2)]
                for jt in range(2):
                    ps = ppa.tile([128, D], F32, tag="acc", name="acc")
                    for k in range(KD):
                        nc.tensor.matmul(ps[:], xT[k][:, 128 * jt:128 * (jt + 1)],
                                         wv_sb[l][k][:], start=(k == 0), stop=(k == KD - 1))
                    nc.scalar.copy(Vn[jt][:], ps[:])
                Qhm = [st.tile([128, n_q * 8], BF16, tag=f"Qhm{k}", name=f"Qhm{k}") for k in range(KD)]
                for nt in range(KD):
                    ps = ppa.tile([128, n_q], F32, tag="acc", name="acc")
                    for k in range(KD):
                        nc.tensor.matmul(ps[:], wq_sb[l][k][:, 128 * nt:128 * (nt + 1)],
                                         xT[k][:, :n_q], start=(k == 0), stop=(k == KD - 1))
                    pstep = ps[:].ap[0][0]
                    in0 = _ap(ps, 0, [[pstep, 128], [1, n_q], [0, 8]])
                    in1 = _ap(cbf_sb, X_HM + nt * 8, [[cbf_pitch, 128], [0, n_q], [1, 8]])
                    outap = _ap(Qhm[nt], 0, [[Qhm[nt][:].ap[0][0], 128], [8, n_q], [1, 8]])
                    nc.vector.scalar_tensor_tensor(
                        out=outap, in0=in0, scalar=bq_sb[:, l * KD + nt:l * KD + nt + 1],
                        in1=in1, op0=OP.add, op1=OP.mult)

                # ke table -> replicated ke_rep [nd][128, 128]
                ke_rep = [st.tile([128, 128], BF16, tag=f"ker{k}", name=f"ker{k}") for k in range(KD)]
                for nt in range(KD):
                    ps = ppq.tile([128, ET], F32, tag="qe", name="qe")
                    for k in range(KD):
                        nc.tensor.matmul(ps[:], wk_sb[l][k][:, 128 * nt:128 * (nt + 1)],
                                         eeT_sb[k][:], start=(k == 0), stop=(k == KD - 1))
                    keT = pw.tile([128, ET], BF16, tag="keT", name="keT")
                    nc.vector.tensor_scalar(out=keT[:], in0=ps[:],
                                            scalar1=bk_sb[:, l * KD + nt:l * KD + nt + 1],
                                            scalar2=None, op0=OP.add)
                    kp = keT[:].ap[0][0]
                    in_ = _ap(keT, 0, [[kp, 128], [0, 8], [1, ET]])
                    outap = _ap(ke_rep[nt], 0, [[ke_rep[nt][:].ap[0][0], 128], [ET, 8], [1, ET]])
                    nc.vector.tensor_copy(outap, in_)

                # ---------- qe (edge-score lhsT), batched over 8 subgroups ----
                # bd_all[:, 64*sg:64*(sg+1)] is the bd-masked lhsT for sg.
                NSGl = n_q // 8
                bd_all = st.tile([128, NSGl * 64], BF16, tag="bdall", name="bdall")
                if QE_BATCH:
                    for ch in range(NSGl // 8):
                        qe_ps = ppq.tile([128, 512], F32, tag="qe", name="qe")
                        for k in range(KD):
                            nc.tensor.matmul(qe_ps[:], ke_rep[k][:],
                                             Qhm[k][:, 512 * ch:512 * (ch + 1)],
                                             start=(k == 0), stop=(k == KD - 1))
                        in1 = _ap(cbf_sb, X_BDM, [[cbf_pitch, 128], [0, 8], [1, 64]])
                        nc.vector.tensor_tensor(out=bd_all[:, 512 * ch:512 * (ch + 1)],
                                                in0=qe_ps[:], in1=in1, op=OP.mult)
                else:
                    for g2 in range(NSGl // 2):
                        qe_ps = ppq.tile([128, 128], F32, tag="qe", name="qe")
                        for s2 in range(2):
                            sg = 2 * g2 + s2
                            for k in range(KD):
                                nc.tensor.matmul(qe_ps[:, 64 * s2:64 * (s2 + 1)],
                                                 ke_rep[k][:], Qhm[k][:, 64 * sg:64 * (sg + 1)],
                                                 start=(k == 0), stop=(k == KD - 1))
                        nc.vector.tensor_tensor(out=bd_all[:, 128 * g2:128 * (g2 + 1)],
                                                in0=qe_ps[:], in1=bdm_sb[:], op=OP.mult)

                # ---------- attention groups ----------
                PT_all = [st.tile([128, NG * 128], BF16, tag=f"PT{j}", name=f"PT{j}") for j in range(2)]
                for g in range(NG):
                    s_ps = pps.tile([128, 256], F32, tag="s", name="s")
                    for k in range(KD):
                        nc.tensor.matmul(s_ps[:], Qhm[k][:, 128 * g:128 * (g + 1)], KT[k][:],
                                         start=(k == 0), stop=False)
                    for s2 in range(2):
                        sg = 2 * g + s2
                        nc.tensor.matmul(s_ps[64 * s2:64 * (s2 + 1), :],
                                         bd_all[:, 64 * sg:64 * (sg + 1)],
                                         oh_sb[:, 256 * sg:256 * (sg + 1)],
                                         start=False, stop=False,
                                         tile_position=(0, 64 * s2))
                    nc.tensor.matmul(s_ps[:], rr_sb[:], mbr_sb[:, 256 * g:256 * (g + 1)],
                                     start=False, stop=True)

                    Pn = pw.tile([128, 256], BF16, tag="Pn", name="Pn")
                    rsum = pw.tile([128, 1], F32, tag="rsum", name="rsum")
                    nc.scalar.activation(Pn[:], s_ps[:], AF.Exp, scale=SCALE,
                                         accum_out=rsum[:])
                    rrec = pw.tile([128, 1], F32, tag="rrec", name="rrec")
                    nc.vector.reciprocal(rrec[:], rsum[:])
                    if DIAG_FOLD:
                        # diag(1/rowsum) folds the softmax normalize into the
                        # P-transpose, done as an explicit matmul so the rhs
                        # values are used: out[j,i] = sum_c Pn[c,j]*diag[c,i]
                        # = Pn[i,j] * rrec[i].  (tensor.transpose ignores the
                        # matrix operand — it uses a dedicated datapath.)
                        diag = pw.tile([128, 128], BF16, tag="diag", name="diag")
                        nc.vector.tensor_scalar(out=diag[:], in0=idb_sb[:],
                                                scalar1=rrec[:, 0:1], scalar2=None,
                                                op0=OP.mult)
                        for jt in range(2):
                            tp = ppt.tile([128, 128], F32, tag="tp", name="tp")
                            nc.tensor.matmul(tp[:], Pn[:, 128 * jt:128 * (jt + 1)],
                                             diag[:], start=True, stop=True)
                            nc.vector.tensor_copy(PT_all[jt][:, 128 * g:128 * (g + 1)], tp[:])
                    else:
                        nc.vector.tensor_scalar(out=Pn[:], in0=Pn[:], scalar1=rrec[:, 0:1],
                                                scalar2=None, op0=OP.mult)
                        for jt in range(2):
                            tp = ppt.tile([128, 128], BF16, tag="tp", name="tp")
                            nc.tensor.transpose(tp[:], Pn[:, 128 * jt:128 * (jt + 1)], idb_sb[:])
                            nc.vector.tensor_copy(PT_all[jt][:, 128 * g:128 * (g + 1)], tp[:])

                # ---------- context ----------
                ctxT = [st.tile([128, n_q], BF16, tag=f"ctxT{d}", name=f"ctxT{d}") for d in range(2)]
                for dt in range(2):
                    cps = ppa.tile([128, n_q], F32, tag="acc", name="acc")
                    for h4 in range(4):
                        h = dt * 4 + h4
                        for jt in range(2):
                            rhs = _ap(PT_all[jt], h,
                                      [[PT_all[jt][:].ap[0][0], 128], [128, NG], [8, 16]])
                            nc.tensor.matmul(cps[32 * h4:32 * (h4 + 1), :],
                                             Vn[jt][:, DH * h:DH * (h + 1)], rhs,
                                             start=(jt == 0), stop=(jt == 1),
                                             tile_position=(0, 32 * h4))
                    nc.scalar.copy(ctxT[dt][:], cps[:])

                # ---------- out-projection + residual + LN1 ----------
                x1 = [st.tile([128, D], F32, tag=f"x1{it}", name=f"x1{it}") for it in range(n_it)]
                for it in range(n_it):
                    ps = ppa.tile([128, D], F32, tag="acc", name="acc")
                    for dt in range(2):
                        nc.tensor.matmul(ps[:], ctxT[dt][:, 128 * it:128 * (it + 1)],
                                         wo_sb[l][dt][:], start=(dt == 0), stop=False)
                    nc.tensor.matmul(ps[:], ones_sb[:], boeff_sb[l][:],
                                     start=False, stop=True)
                    xatt = pw.tile([128, D], F32, tag="xatt", name="xatt")
                    nc.vector.tensor_tensor(out=xatt[:], in0=ps[:], in1=x_nat[it][:], op=OP.add)
                    _layernorm(nc, pw, xatt, x1[it], ln_sb[l][0], ln_sb[l][1], eps_sb)

                # ---------- FFN ----------
                x1T = [st.tile([128, n_q], BF16, tag=f"x1T{k}", name=f"x1T{k}") for k in range(KD)]
                for k in range(KD):
                    for it in range(n_it):
                        tp = ppt.tile([128, 128], F32, tag="tp", name="tp")
                        nc.tensor.transpose(tp[:], x1[it][:, 128 * k:128 * (k + 1)], idf_sb[:])
                        nc.scalar.copy(x1T[k][:, 128 * it:128 * (it + 1)], tp[:])
                hT = [st.tile([128, n_q], BF16, tag=f"hT{f}", name=f"hT{f}") for f in range(FT)]
                for ft in range(FT):
                    ps = ppa.tile([128, n_q], F32, tag="acc", name="acc")
                    for k in range(KD):
                        nc.tensor.matmul(ps[:], w1_sb[l][k][:, 128 * ft:128 * (ft + 1)],
                                         x1T[k][:], start=(k == 0), stop=(k == KD - 1))
                    nc.vector.tensor_scalar(out=hT[ft][:], in0=ps[:],
                                            scalar1=b1T_sb[:, l * FT + ft:l * FT + ft + 1],
                                            scalar2=0.0, op0=OP.add, op1=OP.max)
                xo = [st.tile([128, D], F32, tag=f"xo{it}", name=f"xo{it}") for it in range(n_it)]
                for it in range(n_it):
                    ps = ppa.tile([128, D], F32, tag="acc", name="acc")
                    for ft in range(FT):
                        nc.tensor.matmul(ps[:], hT[ft][:, 128 * it:128 * (it + 1)],
                                         w2_sb[l][ft][:], start=(ft == 0), stop=False)
                    nc.tensor.matmul(ps[:], ones_sb[:], b2r_sb[l][:],
                                     start=False, stop=True)
                    x2pre = pw.tile([128, D], F32, tag="x2pre", name="x2pre")
                    nc.vector.tensor_tensor(out=x2pre[:], in0=ps[:], in1=x1[it][:], op=OP.add)
                    _layernorm(nc, pw, x2pre, xo[it], ln_sb[l][2], ln_sb[l][3], eps_sb)
                x_nat = xo

            nc.sync.dma_start(out_d[:], x_nat[0][:])

    nc.compile()
    return nc


def _layernorm(nc, pw, xin, xout, lns_bc, lnb_bc, eps_sb):
    """LN over the free dim: xout = (xin - mean)/sqrt(var+eps) * lns + lnb."""
    st6 = pw.tile([128, 6], F32, tag="st6", name="st6")
    nc.vector.bn_stats(st6[:], xin[:])
    st2 = pw.tile([128, 2], F32, tag="st2", name="st2")
    nc.vector.bn_aggr(st2[:], st6[:])
    std = pw.tile([128, 1], F32, tag="std", name="std")
    nc.scalar.activation(std[:], st2[:, 1:2], AF.Sqrt, bias=eps_sb[:, 0:1])
    rstd = pw.tile([128, 1], F32, tag="rstd", name="rstd")
    nc.vector.reciprocal(rstd[:], std[:])
    u = pw.tile([128, D], F32, tag="lnu", name="lnu")
    nc.vector.scalar_tensor_tensor(out=u[:], in0=xin[:], scalar=st2[:, 0:1],
                                   in1=lns_bc[:], op0=OP.subtract, op1=OP.mult)
    nc.vector.scalar_tensor_tensor(out=xout[:], in0=u[:], scalar=rstd[:, 0:1],
                                   in1=lnb_bc[:], op0=OP.mult, op1=OP.add)


def prep_inputs(inputs):
    """Host-side sharding/layout prep. Returns per-core input dicts."""
    f32 = np.float32
    tok_emb = np.ascontiguousarray(inputs["tok_emb"], f32)
    pos_emb = np.asarray(inputs["pos_emb"], f32)
    edge_emb = np.asarray(inputs["edge_emb"], f32)
    word_ids = np.asarray(inputs["word_ids"])
    adj = np.asarray(inputs["adj"])
    edge_types = np.asarray(inputs["edge_types"])

    Wq = np.asarray(inputs["Wq"], f32)
    Wk = np.asarray(inputs["Wk"], f32)
    Wv = np.asarray(inputs["Wv"], f32)
    Wo = np.asarray(inputs["Wo"], f32)
    W1 = np.asarray(inputs["W1"], f32)
    W2 = np.asarray(inputs["W2"], f32)
    bqv = np.asarray(inputs["bq"], f32)
    bkv = np.asarray(inputs["bk"], f32)
    bvv = np.asarray(inputs["bv"], f32)
    bov = np.asarray(inputs["bo"], f32)
    b1v = np.asarray(inputs["b1"], f32)
    b2v = np.asarray(inputs["b2"], f32)

    shared = {}

    # wA: [wq0 | wk0 | wv0 | eeT]
    wA = np.empty((128, WA_W), f32)
    for k in range(KD):
        wA[:, A_WQ + 256 * k:A_WQ + 256 * (k + 1)] = Wq[0][128 * k:128 * (k + 1), :]
        wA[:, A_WK + 256 * k:A_WK + 256 * (k + 1)] = Wk[0][128 * k:128 * (k + 1), :]
        wA[:, A_WV + 256 * k:A_WV + 256 * (k + 1)] = Wv[0][128 * k:128 * (k + 1), :]
        wA[:, A_EE + ET * k:A_EE + ET * (k + 1)] = edge_emb.T[128 * k:128 * (k + 1), :]
    shared["wA"] = wA.astype(bf16)

    # wB: [wo0 | w1_0 | w2_0]
    wB = np.empty((128, WB_W), f32)
    for k in range(KD):
        wB[:, B_WO + 256 * k:B_WO + 256 * (k + 1)] = Wo[0][128 * k:128 * (k + 1), :]
        wB[:, B_W1 + 1024 * k:B_W1 + 1024 * (k + 1)] = W1[0][128 * k:128 * (k + 1), :]
    for f in range(FT):
        wB[:, B_W2 + 256 * f:B_W2 + 256 * (f + 1)] = W2[0][128 * f:128 * (f + 1), :]
    shared["wB"] = wB.astype(bf16)

    # wC: layer-1 weights
    wC = np.empty((128, WC_W), f32)
    for k in range(KD):
        wC[:, C_WQ + 256 * k:C_WQ + 256 * (k + 1)] = Wq[1][128 * k:128 * (k + 1), :]
        wC[:, C_WK + 256 * k:C_WK + 256 * (k + 1)] = Wk[1][128 * k:128 * (k + 1), :]
        wC[:, C_WV + 256 * k:C_WV + 256 * (k + 1)] = Wv[1][128 * k:128 * (k + 1), :]
        wC[:, C_WO + 256 * k:C_WO + 256 * (k + 1)] = Wo[1][128 * k:128 * (k + 1), :]
        wC[:, C_W1 + 1024 * k:C_W1 + 1024 * (k + 1)] = W1[1][128 * k:128 * (k + 1), :]
    for f in range(FT):
        wC[:, C_W2 + 256 * f:C_W2 + 256 * (f + 1)] = W2[1][128 * f:128 * (f + 1), :]
    shared["wC"] = wC.astype(bf16)

    # cst: [bq | bk | b1T | idf]
    cst = np.zeros((128, CST_W), f32)
    cst[:, F_BQ:F_BQ + L * KD] = bqv.reshape(L, KD, 128).transpose(2, 0, 1).reshape(128, L * KD)
    cst[:, F_BK:F_BK + L * KD] = bkv.reshape(L, KD, 128).transpose(2, 0, 1).reshape(128, L * KD)
    cst[:, F_B1T:F_B1T + L * FT] = b1v.reshape(L, FT, 128).transpose(2, 0, 1).reshape(128, L * FT)
    cst[:, F_IDF:F_IDF + 128] = np.eye(128, dtype=f32)
    shared["cst"] = cst
    lnp = np.stack([np.asarray(inputs["ln1_s"], f32), np.asarray(inputs["ln1_b"], f32),
                    np.asarray(inputs["ln2_s"], f32), np.asarray(inputs["ln2_b"], f32)],
                   axis=1).reshape(L * 4, D)
    shared["lnr"] = np.ascontiguousarray(
        np.broadcast_to(lnp.reshape(1, LNR_W), (128, LNR_W)))

    # cbf: [hm | bdm | idb]
    cbf = np.zeros((128, CBF_W), f32)
    for nt in range(KD):
        for p in range(128):
            cbf[p, X_HM + nt * 8 + ((nt * 128 + p) // DH)] = 1.0
    for p in range(128):
        for c in range(128):
            if p // 16 == (c % 64) // 8:
                cbf[p, X_BDM + c] = 1.0
    cbf[:, X_IDB:X_IDB + 128] = np.eye(128, dtype=f32)
    shared["cbf"] = cbf.astype(bf16)

    boeff = bov + bvv @ Wo  # [L, D]
    shared["boeff"] = boeff.astype(bf16)
    shared["b2r"] = b2v.astype(bf16)

    in_maps = []
    for core in range(N_CORES):
        b, half = core // 2, core % 2
        own = np.arange(half * 128, half * 128 + 128)
        other = np.arange((1 - half) * 128, (1 - half) * 128 + 128)
        perm = np.concatenate([own, other])
        m = dict(shared)
        m["x0"] = np.ascontiguousarray(
            (tok_emb[word_ids[b][perm]] + pos_emb[:N][perm]).reshape(2, 128, D))
        adj_l = adj[b][np.ix_(perm, perm)]
        mb = np.where(adj_l > 0, 0.0, -30000.0).astype(f32)
        # m16: [mbr | rr];  mbr[p, g*256+j] = mb[16g+p, j]
        m16 = np.zeros((16, M16_W), f32)
        m16[:, M_MBR:M_MBR + 4096] = mb.reshape(16, 16, 256).transpose(1, 0, 2).reshape(16, 16 * 256)
        for r in range(16):
            m16[r, M_RR + r * 8:M_RR + (r + 1) * 8] = 1.0
        m["m16"] = m16.astype(bf16)
        # oh[p=16r+t, sg*256+j] = (edge_types[8sg+r, j] == t), over permuted graph
        et = edge_types[b][np.ix_(perm, perm)]
        et_r = et.reshape(N // 8, 8, N).transpose(1, 0, 2)          # [r, sg, j]
        ohm = (et_r[:, None, :, :] == np.arange(ET)[None, :, None, None])
        m["oh"] = ohm.reshape(128, OH_W).astype(bf16)
        in_maps.append(m)
    return in_maps


_NC_CACHE = {}


def get_nc():
    if "nc" not in _NC_CACHE:
        _NC_CACHE["nc"] = build_nc()
    return _NC_CACHE["nc"]


def kernel(**inputs):
    nc = get_nc()
    in_maps = prep_inputs(inputs)
    res = run_bass_kernel_spmd(nc, in_maps, list(range(N_CORES)))
    out = np.zeros((B, N, D), np.float32)
    for core in range(N_CORES):
        b, half = core // 2, core % 2
        out[b, half * 128:half * 128 + 128] = res.results[core]["out"]
    return out
